# revision 29
# baseline (speedup 1.0000x reference)
"""Trainium2 kernel for nn_EntropyAndMutualInformation.

reference:
    probs_X = softmax(act_X, axis=1); probs_Y = softmax(act_Y, axis=1)
    entropy_X = -mean_b sum_d probs_X^2
    entropy_Y = -mean_b sum_d probs_Y^2
    mi = mean_b sum_{i,j} (probs_X[b,i] * probs_Y[b,j])^2

Because sum_{i,j}(p_i q_j)^2 = (sum_i p_i^2)(sum_j q_j^2), the [B,D,D]
joint never needs materializing. With sp2[b] = sum_d softmax(row b)^2:
    entropy_X = -mean(sp2_X), entropy_Y = -mean(sp2_Y),
    mi = mean(sp2_X * sp2_Y).

Sharding: data-parallel over B=2048 -> 8 cores x 256 rows, identical
SPMD program per core; the 3 scalars are reduced on host from the raw
bn_stats records each core emits.

Device program (raw engine streams, no nc.Block). The NEFF's fixed
epilogue (walrus' per-semaphore clear flood, ~7us Tensor-bound) starts
once every engine reaches the end-of-program barrier, so the kernel is
shaped to get the last engine there as early as possible:
  - inputs move as row-half DMAs (full 2048 B row descriptors -- DMA
    engines are descriptor-rate-bound at ~80ns/desc, so column chunking
    multiplies wire time) spread over three queues: Sync HWDGE gets
    X0 + the column-split tail halves Y1a/Y1b, GpSimd SWDGE gets Y0/X1
    (descriptor prep runs on the otherwise idle pool core)
  - Scalar: dummy Exp first so the auto-inserted ACT table load (~1.5us)
    runs inside the DMA window, then one Exp per chunk as data lands
  - Vector: bn_stats per exp'd chunk (raw even/odd count/mean/n*var
    records; host aggregates)
  - Sync: fire-and-forget DMA of the stats block -- no completion wait;
    the walrus teardown's DGE drain fences the transfer long before the
    epilogue ends
"""

from contextlib import ExitStack

import numpy as np

import concourse.bass as bass
from concourse import mybir
from concourse.bass_utils import run_bass_kernel_spmd

B = 2048
D = 512
N_CORES = 8
ROWS = B // N_CORES  # 256
P = 128
HD = D // 2  # 256


def build_nc() -> bass.Bass:
    nc = bass.Bass()
    x = nc.declare_dram_parameter("act_X", [ROWS, D], mybir.dt.float32, isOutput=False)
    y = nc.declare_dram_parameter("act_Y", [ROWS, D], mybir.dt.float32, isOutput=False)
    out = nc.declare_dram_parameter("out", [P, 36], mybir.dt.float32, isOutput=True)

    zero = nc.const_aps.aps[(mybir.dt.float32, 0.0)]

    with ExitStack() as ctx:
        # inputs and exp outputs as planes of single tensors, order
        # X0,Y0,X1,Y1 -- X1/Y1 adjacent so both 64-col tails exp in ONE
        # 3D-AP activation (elementwise; walrus accepts it, unlike bn_stats)
        tt = ctx.enter_context(nc.sbuf_tensor("tt", [P, 4, D], mybir.dt.float32))
        ee = ctx.enter_context(nc.sbuf_tensor("ee", [P, 4, D], mybir.dt.float32))
        warm = ctx.enter_context(nc.sbuf_tensor("warm", [P, 1], mybir.dt.float32))
        stats = ctx.enter_context(nc.sbuf_tensor("stats", [P, 6, 6], mybir.dt.float32))

        sd = [ctx.enter_context(nc.semaphore(f"sd{i}")) for i in range(7)]
        sa = ctx.enter_context(nc.semaphore("sa"))
        sbn = ctx.enter_context(nc.semaphore("sbn"))
        so = ctx.enter_context(nc.semaphore("so"))

        # Scalar queue carries exactly ONE input chunk (Y0 left 3/4, 192 KB),
        # issued before the dummy Exp: the ACT table load stalls this queue
        # while it runs, but the chunk still clears before its exp needs it.
        # The 64 KB right quarter rides GpSimd's slow SWDGE queue, which this
        # split also unloads so Y1a lands ~0.7us earlier.
        HC = 384
        nc.scalar.dma_start(out=tt[:, 1, 0:HC], in_=y[0:P, 0:HC]).then_inc(sd[6], 16)

        # dummy Exp: hoists the ACT table load here, inside the DMA window
        nc.scalar.activation(
            out=warm[:, :],
            in_=zero,
            func=mybir.ActivationFunctionType.Exp,
            bias=zero,
            scale=1.0,
        )

        # chunk cascade (exp/bn slots follow it); both late halves are
        # column-split so no full-width Exp sits at the end of the chain:
        #   0: X half0          (Sync queue head, full rows)
        #   1: Y half0          (GpSimd SWDGE head, full rows)
        #   2: X half1 [0:448)  (Sync queue)
        #   3: Y half1 [0:448)  (GpSimd SWDGE)
        #   4: X half1 [448:512)(Sync queue)    <- tiny tail
        #   5: Y half1 [448:512)(GpSimd SWDGE)  <- tiny tail
        # GpSimd issues its tiny Y tail FIRST: its ~0.65us SWDGE prep delays
        # Y half0's transfer start, so X half0 (the pipeline head on the
        # Sync queue) gets the DMA engines to itself early. Chunk sems are
        # independent, so arrival order never deadlocks the exp chain.
        TC = 448  # tail split point
        nc.sync.dma_start(out=tt[:, 0, :], in_=x[0:P, :]).then_inc(sd[0], 16)
        nc.gpsimd.dma_start(out=tt[:, 3, TC:D], in_=y[P:ROWS, TC:D]).then_inc(sd[5], 16)
        nc.gpsimd.dma_start(out=tt[:, 1, HC:D], in_=y[0:P, HC:D]).then_inc(sd[1], 16)
        nc.sync.dma_start(out=tt[:, 2, 0:TC], in_=x[P:ROWS, 0:TC]).then_inc(sd[2], 16)
        nc.gpsimd.dma_start(out=tt[:, 3, 0:TC], in_=y[P:ROWS, 0:TC]).then_inc(sd[3], 16)
        nc.sync.dma_start(out=tt[:, 2, TC:D], in_=x[P:ROWS, TC:D]).then_inc(sd[4], 16)

        # exp ops; the last is ONE op over both 64-col tails (planes 2:4)
        exp_plan = [
            ([sd[0]], tt[:, 0, :], ee[:, 0, :]),
            ([sd[6], sd[1]], tt[:, 1, :], ee[:, 1, :]),
            ([sd[2]], tt[:, 2, 0:TC], ee[:, 2, 0:TC]),
            ([sd[3]], tt[:, 3, 0:TC], ee[:, 3, 0:TC]),
            ([sd[4], sd[5]], tt[:, 2:4, TC:D], ee[:, 2:4, TC:D]),
        ]
        for sems, src, dst in exp_plan:
            for sem in sems:
                nc.scalar.wait_ge(sem, 16)
            nc.scalar.activation(
                out=dst,
                in_=src,
                func=mybir.ActivationFunctionType.Exp,
                bias=zero,
                scale=1.0,
            ).then_inc(sa, 1)

        # bn slots 0..5; the two tails (ops 4,5) are both gated on the
        # merged tail exp (sa >= 5)
        bn_plan = [
            (1, ee[:, 0, :], stats[:, 0, :]),
            (2, ee[:, 1, :], stats[:, 1, :]),
            (3, ee[:, 2, 0:TC], stats[:, 2, :]),
            (4, ee[:, 3, 0:TC], stats[:, 3, :]),
            (5, ee[:, 2, TC:D], stats[:, 4, :]),
            (5, ee[:, 3, TC:D], stats[:, 5, :]),
        ]
        for k, (need, src, dst) in enumerate(bn_plan):
            nc.vector.wait_ge(sa, need)
            ins = nc.vector.bn_stats(out=dst, in_=src)
            if k == len(bn_plan) - 1:
                ins.then_inc(sbn, 1)

        # fire-and-forget: no completion wait; the walrus teardown's DGE
        # drain fences the transfer well inside the ~7us epilogue
        nc.sync.wait_ge(sbn, 1)
        nc.sync.dma_start(
            out=out[:, :], in_=stats[:, :, :], single_packet=True
        ).then_inc(so, 16)

    nc.finalize()
    return nc


_NC_CACHE: bass.Bass | None = None


def _get_nc() -> bass.Bass:
    global _NC_CACHE
    if _NC_CACHE is None:
        _NC_CACHE = build_nc()
    return _NC_CACHE


def _sp2_from_stats(o: np.ndarray) -> tuple[np.ndarray, np.ndarray]:
    """[128, 36] raw bn_stats -> (sp2_x[256], sp2_y[256]) in shard row order."""
    o = np.asarray(o, dtype=np.float64).reshape(P, 6, 6)
    s1 = np.empty((P, 6))
    s2 = np.empty((P, 6))
    for j in range(6):
        ne, me, nve, no, mo, nvo = (o[:, j, k] for k in range(6))
        s1[:, j] = ne * me + no * mo
        s2[:, j] = nve + nvo + ne * me * me + no * mo * mo
    # slots: 0 = X rows 0:128, 1 = Y rows 0:128,
    #        2 + 4 = X rows 128:256 (column-split), 3 + 5 = Y rows 128:256
    sp2x = np.concatenate(
        [s2[:, 0] / s1[:, 0] ** 2, (s2[:, 2] + s2[:, 4]) / (s1[:, 2] + s1[:, 4]) ** 2]
    )
    sp2y = np.concatenate(
        [s2[:, 1] / s1[:, 1] ** 2, (s2[:, 3] + s2[:, 5]) / (s1[:, 3] + s1[:, 5]) ** 2]
    )
    return sp2x, sp2y


def run_sharded(act_X: np.ndarray, act_Y: np.ndarray, **spmd_kwargs):
    """Shard over B, run on 8 cores; returns (output[3] f32, BassKernelResults)."""
    act_X = np.ascontiguousarray(act_X, dtype=np.float32)
    act_Y = np.ascontiguousarray(act_Y, dtype=np.float32)
    assert act_X.shape == (B, D) and act_Y.shape == (B, D)

    in_maps = [
        {
            "act_X": act_X[i * ROWS : (i + 1) * ROWS],
            "act_Y": act_Y[i * ROWS : (i + 1) * ROWS],
        }
        for i in range(N_CORES)
    ]
    # the runtime occasionally throws a transient NRT exec-unit error that
    # clears on the next execution; retry a couple of times before giving up
    last_err = None
    for _ in range(3):
        try:
            br = run_bass_kernel_spmd(
                _get_nc(), in_maps, list(range(N_CORES)), **spmd_kwargs
            )
            break
        except Exception as e:  # noqa: BLE001
            last_err = e
    else:
        raise last_err

    sxs, sys_ = [], []
    for i in range(N_CORES):
        sp2x, sp2y = _sp2_from_stats(br.results[i]["out"])
        sxs.append(sp2x)
        sys_.append(sp2y)
    sx = np.concatenate(sxs)
    sy = np.concatenate(sys_)

    out = np.array([-sx.mean(), -sy.mean(), (sx * sy).mean()], dtype=np.float32)
    return out, br


def kernel(act_X: np.ndarray, act_Y: np.ndarray) -> np.ndarray:
    out, _ = run_sharded(act_X, act_Y)
    return out


# revision 31
# speedup vs baseline: 1.0127x; 1.0127x over previous
"""Trainium2 kernel for nn_EntropyAndMutualInformation.

reference:
    probs_X = softmax(act_X, axis=1); probs_Y = softmax(act_Y, axis=1)
    entropy_X = -mean_b sum_d probs_X^2
    entropy_Y = -mean_b sum_d probs_Y^2
    mi = mean_b sum_{i,j} (probs_X[b,i] * probs_Y[b,j])^2

Because sum_{i,j}(p_i q_j)^2 = (sum_i p_i^2)(sum_j q_j^2), the [B,D,D]
joint never needs materializing. With sp2[b] = sum_d softmax(row b)^2:
    entropy_X = -mean(sp2_X), entropy_Y = -mean(sp2_Y),
    mi = mean(sp2_X * sp2_Y).

Sharding: data-parallel over B=2048 -> 8 cores x 256 rows, identical
SPMD program per core; the 3 scalars are reduced on host from the raw
bn_stats records each core emits.

Device program (raw engine streams, no nc.Block). The NEFF's fixed
epilogue (walrus' per-semaphore clear flood, ~7us Tensor-bound) starts
once every engine reaches the end-of-program barrier, so the kernel is
shaped to get the last engine there as early as possible:
  - inputs move as row-half DMAs (full 2048 B row descriptors -- DMA
    engines are descriptor-rate-bound at ~80ns/desc, so column chunking
    multiplies wire time) spread over three queues: Sync HWDGE gets
    X0 + the column-split tail halves Y1a/Y1b, GpSimd SWDGE gets Y0/X1
    (descriptor prep runs on the otherwise idle pool core)
  - Scalar: dummy Exp first so the auto-inserted ACT table load (~1.5us)
    runs inside the DMA window, then one Exp per chunk as data lands
  - Vector: bn_stats per exp'd chunk (raw even/odd count/mean/n*var
    records; host aggregates)
  - Sync: fire-and-forget DMA of the stats block -- no completion wait;
    the walrus teardown's DGE drain fences the transfer long before the
    epilogue ends
"""

from contextlib import ExitStack

import numpy as np

import concourse.bass as bass
from concourse import mybir
from concourse.bass_utils import run_bass_kernel_spmd

B = 2048
D = 512
N_CORES = 8
ROWS = B // N_CORES  # 256
P = 128
HD = D // 2  # 256


def build_nc() -> bass.Bass:
    nc = bass.Bass()
    x = nc.declare_dram_parameter("act_X", [ROWS, D], mybir.dt.float32, isOutput=False)
    y = nc.declare_dram_parameter("act_Y", [ROWS, D], mybir.dt.float32, isOutput=False)
    out = nc.declare_dram_parameter("out", [P, 36], mybir.dt.float32, isOutput=True)

    zero = nc.const_aps.aps[(mybir.dt.float32, 0.0)]

    with ExitStack() as ctx:
        # inputs and exp outputs as planes of single tensors, order
        # X0,Y0,X1,Y1 -- X1/Y1 adjacent so both 64-col tails exp in ONE
        # 3D-AP activation (elementwise; walrus accepts it, unlike bn_stats)
        tt = ctx.enter_context(nc.sbuf_tensor("tt", [P, 4, D], mybir.dt.float32))
        ee = ctx.enter_context(nc.sbuf_tensor("ee", [P, 4, D], mybir.dt.float32))
        warm = ctx.enter_context(nc.sbuf_tensor("warm", [P, 1], mybir.dt.float32))
        stats = ctx.enter_context(nc.sbuf_tensor("stats", [P, 6, 6], mybir.dt.float32))

        sd = [ctx.enter_context(nc.semaphore(f"sd{i}")) for i in range(7)]
        sa = ctx.enter_context(nc.semaphore("sa"))
        sbn = ctx.enter_context(nc.semaphore("sbn"))
        so = ctx.enter_context(nc.semaphore("so"))

        # Scalar queue carries exactly ONE small input chunk (Y0 left half,
        # 128 KB), issued before the dummy Exp: the ACT table load stalls
        # this queue while it runs, but a 128 KB chunk still clears ~1us
        # before its exp needs it (224 KB chunks here regressed in v6).
        HC = 256
        nc.scalar.dma_start(out=tt[:, 1, 0:HC], in_=y[0:P, 0:HC]).then_inc(sd[6], 16)

        # dummy Exp: hoists the ACT table load here, inside the DMA window
        nc.scalar.activation(
            out=warm[:, :],
            in_=zero,
            func=mybir.ActivationFunctionType.Exp,
            bias=zero,
            scale=1.0,
        )

        # chunk cascade (exp/bn slots follow it); both late halves are
        # column-split so no full-width Exp sits at the end of the chain:
        #   0: X half0          (Sync queue head, full rows)
        #   1: Y half0          (GpSimd SWDGE head, full rows)
        #   2: X half1 [0:448)  (Sync queue)
        #   3: Y half1 [0:448)  (GpSimd SWDGE)
        #   4: X half1 [448:512)(Sync queue)    <- tiny tail
        #   5: Y half1 [448:512)(GpSimd SWDGE)  <- tiny tail
        # GpSimd issues its tiny Y tail FIRST: its ~0.65us SWDGE prep delays
        # Y half0's transfer start, so X half0 (the pipeline head on the
        # Sync queue) gets the DMA engines to itself early. Chunk sems are
        # independent, so arrival order never deadlocks the exp chain.
        # Y0's right half rides the Sync queue as chunk #2: the GpSimd SWDGE
        # queue's lag is intrinsic (v12: even 64 KB there lands ~+5.8), and
        # Sync's #2 slot delivers ~+4.0, eliminating the ~1.2us ACT idle
        # before exp2. GpSimd keeps only the tiny Y tail + Y1a.
        TC = 448  # tail split point
        nc.sync.dma_start(out=tt[:, 0, :], in_=x[0:P, :]).then_inc(sd[0], 16)
        nc.gpsimd.dma_start(out=tt[:, 3, TC:D], in_=y[P:ROWS, TC:D]).then_inc(sd[5], 16)
        nc.sync.dma_start(out=tt[:, 1, HC:D], in_=y[0:P, HC:D]).then_inc(sd[1], 16)
        nc.sync.dma_start(out=tt[:, 2, 0:TC], in_=x[P:ROWS, 0:TC]).then_inc(sd[2], 16)
        nc.gpsimd.dma_start(out=tt[:, 3, 0:TC], in_=y[P:ROWS, 0:TC]).then_inc(sd[3], 16)
        nc.sync.dma_start(out=tt[:, 2, TC:D], in_=x[P:ROWS, TC:D]).then_inc(sd[4], 16)

        # exp ops; the last is ONE op over both 64-col tails (planes 2:4)
        exp_plan = [
            ([sd[0]], tt[:, 0, :], ee[:, 0, :]),
            ([sd[6], sd[1]], tt[:, 1, :], ee[:, 1, :]),
            ([sd[2]], tt[:, 2, 0:TC], ee[:, 2, 0:TC]),
            ([sd[3]], tt[:, 3, 0:TC], ee[:, 3, 0:TC]),
            ([sd[4], sd[5]], tt[:, 2:4, TC:D], ee[:, 2:4, TC:D]),
        ]
        for sems, src, dst in exp_plan:
            for sem in sems:
                nc.scalar.wait_ge(sem, 16)
            nc.scalar.activation(
                out=dst,
                in_=src,
                func=mybir.ActivationFunctionType.Exp,
                bias=zero,
                scale=1.0,
            ).then_inc(sa, 1)

        # bn slots 0..5; the two tails (ops 4,5) are both gated on the
        # merged tail exp (sa >= 5)
        bn_plan = [
            (1, ee[:, 0, :], stats[:, 0, :]),
            (2, ee[:, 1, :], stats[:, 1, :]),
            (3, ee[:, 2, 0:TC], stats[:, 2, :]),
            (4, ee[:, 3, 0:TC], stats[:, 3, :]),
            (5, ee[:, 2, TC:D], stats[:, 4, :]),
            (5, ee[:, 3, TC:D], stats[:, 5, :]),
        ]
        for k, (need, src, dst) in enumerate(bn_plan):
            nc.vector.wait_ge(sa, need)
            ins = nc.vector.bn_stats(out=dst, in_=src)
            if k == len(bn_plan) - 1:
                ins.then_inc(sbn, 1)

        # fire-and-forget: no completion wait; the walrus teardown's DGE
        # drain fences the transfer well inside the ~7us epilogue
        nc.sync.wait_ge(sbn, 1)
        nc.sync.dma_start(
            out=out[:, :], in_=stats[:, :, :], single_packet=True
        ).then_inc(so, 16)

    nc.finalize()
    return nc


_NC_CACHE: bass.Bass | None = None


def _get_nc() -> bass.Bass:
    global _NC_CACHE
    if _NC_CACHE is None:
        _NC_CACHE = build_nc()
    return _NC_CACHE


def _sp2_from_stats(o: np.ndarray) -> tuple[np.ndarray, np.ndarray]:
    """[128, 36] raw bn_stats -> (sp2_x[256], sp2_y[256]) in shard row order."""
    o = np.asarray(o, dtype=np.float64).reshape(P, 6, 6)
    s1 = np.empty((P, 6))
    s2 = np.empty((P, 6))
    for j in range(6):
        ne, me, nve, no, mo, nvo = (o[:, j, k] for k in range(6))
        s1[:, j] = ne * me + no * mo
        s2[:, j] = nve + nvo + ne * me * me + no * mo * mo
    # slots: 0 = X rows 0:128, 1 = Y rows 0:128,
    #        2 + 4 = X rows 128:256 (column-split), 3 + 5 = Y rows 128:256
    sp2x = np.concatenate(
        [s2[:, 0] / s1[:, 0] ** 2, (s2[:, 2] + s2[:, 4]) / (s1[:, 2] + s1[:, 4]) ** 2]
    )
    sp2y = np.concatenate(
        [s2[:, 1] / s1[:, 1] ** 2, (s2[:, 3] + s2[:, 5]) / (s1[:, 3] + s1[:, 5]) ** 2]
    )
    return sp2x, sp2y


def run_sharded(act_X: np.ndarray, act_Y: np.ndarray, **spmd_kwargs):
    """Shard over B, run on 8 cores; returns (output[3] f32, BassKernelResults)."""
    act_X = np.ascontiguousarray(act_X, dtype=np.float32)
    act_Y = np.ascontiguousarray(act_Y, dtype=np.float32)
    assert act_X.shape == (B, D) and act_Y.shape == (B, D)

    in_maps = [
        {
            "act_X": act_X[i * ROWS : (i + 1) * ROWS],
            "act_Y": act_Y[i * ROWS : (i + 1) * ROWS],
        }
        for i in range(N_CORES)
    ]
    # the runtime occasionally throws a transient NRT exec-unit error that
    # clears on the next execution; retry a couple of times before giving up
    last_err = None
    for _ in range(3):
        try:
            br = run_bass_kernel_spmd(
                _get_nc(), in_maps, list(range(N_CORES)), **spmd_kwargs
            )
            break
        except Exception as e:  # noqa: BLE001
            last_err = e
    else:
        raise last_err

    sxs, sys_ = [], []
    for i in range(N_CORES):
        sp2x, sp2y = _sp2_from_stats(br.results[i]["out"])
        sxs.append(sp2x)
        sys_.append(sp2y)
    sx = np.concatenate(sxs)
    sy = np.concatenate(sys_)

    out = np.array([-sx.mean(), -sy.mean(), (sx * sy).mean()], dtype=np.float32)
    return out, br


def kernel(act_X: np.ndarray, act_Y: np.ndarray) -> np.ndarray:
    out, _ = run_sharded(act_X, act_Y)
    return out
